# revision 15
# baseline (speedup 1.0000x reference)
"""Expert-parallel MoE (Grok-1-style dense-masked MoE layer) for Trainium2.

Strategy (8 NeuronCores, expert-parallel):
  - Router top-2 decisions are computed on the host (fp32, verified to match
    the jax reference's decisions exactly on margins >> fp error), and used
    ONLY to build a per-expert gathered token list (capacity-padded).
  - Each core owns one expert e: it receives x gathered for its tokens
    (xg = x[idx_e].T), the expert's weights, and the full xT (for the
    router-probs output, computed on device).
  - On device, each core computes:
      * router probs over ALL tokens (softmax(x @ gate_w)) -> probs output
        (host keeps core 0's copy),
      * the top-2-renormalized combine weight for its own expert on its
        gathered tokens (softmax + 2nd-max + renorm, all on-chip),
      * h1 = xg @ w_in[e], h2 = xg @ w_v[e]  (f32r matmuls, contraction
        over D on partitions),
      * g = gelu(h1) * h2 kept on SBUF in [DF, tokens] layout,
      * out = (g.T @ w_out[e]) * combine  -> DMA to HBM.
  - Host unshard: output[idx_e] += core_e_out[:count_e]  (the all-reduce
    over experts collapses to a scatter-add since non-routed tokens
    contribute exactly zero).
"""

import numpy as np
from contextlib import ExitStack

import concourse.bass as bass
import concourse.tile as tile
from concourse import bacc, mybir
from concourse.bass import ts, ds
from concourse.bass_utils import run_bass_kernel_spmd

B, S, D, DF, E, TOPK = 2, 1024, 1024, 4096, 8, 2
T = B * S
P = 128
NCORES = 8

F32 = mybir.dt.float32
F32R = mybir.dt.float32r
AX = mybir.AxisListType.X
OP = mybir.AluOpType
ACTF = mybir.ActivationFunctionType


def _slabs(n, mx=512):
    """Split n into contiguous slabs each <= mx, as even as possible."""
    k = -(-n // mx)
    w = -(-n // k)
    out = []
    o = 0
    while o < n:
        s = min(w, n - o)
        out.append((o, s))
        o += s
    return out


def build_moe(nc, C, gelu_func=None):
    """Emit the per-core kernel IR. C = gathered-token capacity (mult of 128)."""
    if gelu_func is None:
        gelu_func = ACTF.Gelu
    assert C % P == 0
    n_tc = T // P      # full-router token chunks (16)
    n_gc = C // P      # gathered token chunks
    DO = D // P        # 8 contraction chunks over D
    NDF = DF // P      # 32 chunks over DF
    NDS = D // 512     # 2 output D slabs

    xT = nc.dram_tensor("xT", [D, T], F32, kind="ExternalInput").ap()
    xg = nc.dram_tensor("xg", [D, C], F32R, kind="ExternalInput").ap()
    w_in = nc.dram_tensor("w_in", [D, DF], F32R, kind="ExternalInput").ap()
    w_v = nc.dram_tensor("w_v", [D, DF], F32R, kind="ExternalInput").ap()
    w_out = nc.dram_tensor("w_out", [DF, D], F32R, kind="ExternalInput").ap()
    gate_w = nc.dram_tensor("gate_w", [D, E], F32, kind="ExternalInput").ap()
    gate_wr = nc.dram_tensor("gate_wr", [D, E], F32R, kind="ExternalInput").ap()
    sel = nc.dram_tensor("sel", [P, 1, E], F32, kind="ExternalInput").ap()
    out = nc.dram_tensor("out", [C, D], F32, kind="ExternalOutput").ap()
    probs = nc.dram_tensor("probs", [T, E], F32, kind="ExternalOutput").ap()

    xT_r = xT.rearrange("(o p) t -> p o t", p=P)
    xg_r = xg.rearrange("(o p) t -> p o t", p=P)
    win_r = w_in.rearrange("(o p) f -> p o f", p=P)
    wv_r = w_v.rearrange("(o p) f -> p o f", p=P)
    wout_r = w_out.rearrange("(o p) d -> p o d", p=P)
    gw_r = gate_w.rearrange("(o p) e -> p o e", p=P)
    gwr_r = gate_wr.rearrange("(o p) e -> p o e", p=P)
    probs_r = probs.rearrange("(t p) e -> p t e", p=P)

    WSLAB = 256 if C <= 1024 else 128
    # token blocks sized so g ([P, NDF, tlen] fp32) fits in SBUF
    if C <= 640:
        blocks = [(0, C)]
    else:
        blocks = [(o, min(512, C - o)) for o in range(0, C, 512)]

    with tile.TileContext(nc) as tc, ExitStack() as ctx:
        const = ctx.enter_context(tc.tile_pool(name="const", bufs=1))
        xgp = ctx.enter_context(tc.tile_pool(name="xgp", bufs=1))
        gp = ctx.enter_context(tc.tile_pool(name="gp", bufs=1))
        winp = ctx.enter_context(tc.tile_pool(name="winp", bufs=2))
        wvp = ctx.enter_context(tc.tile_pool(name="wvp", bufs=2))
        woutp = ctx.enter_context(tc.tile_pool(name="woutp", bufs=3))
        xrp = ctx.enter_context(tc.tile_pool(name="xrp", bufs=3))
        smp = ctx.enter_context(tc.tile_pool(name="smp", bufs=1))
        cmbp = ctx.enter_context(tc.tile_pool(name="cmbp", bufs=1))
        gelp = ctx.enter_context(tc.tile_pool(name="gelp", bufs=2))
        outp = ctx.enter_context(tc.tile_pool(name="outp", bufs=4))
        psum = ctx.enter_context(tc.tile_pool(name="psum", bufs=8, space="PSUM"))

        # ---------------- constants ----------------
        gw_sb = const.tile([P, DO, E], F32, tag="gw")
        nc.sync.dma_start(gw_sb[:], gw_r[:, :, :])
        gwr_sb = const.tile([P, DO, E], F32R, tag="gwr")
        nc.sync.dma_start(gwr_sb[:], gwr_r[:, :, :])
        sel_sb = const.tile([P, 1, E], F32, tag="sel")
        nc.sync.dma_start(sel_sb[:], sel[:, :, :])

        # ---------------- gathered activations (resident) ----------------
        xg_sb = xgp.tile([P, DO, C], F32R, tag="xg")
        nc.sync.dma_start(xg_sb[:], xg_r[:, :, :])

        # ---------------- router over ALL tokens -> probs output ----------
        pr = psum.tile([P, n_tc * E], F32, tag="pp")
        for tci in range(n_tc):
            xr = xrp.tile([P, DO, P], F32, tag="xr")
            nc.sync.dma_start(xr[:], xT_r[:, :, ts(tci, P)])
            for d in range(DO):
                nc.tensor.matmul(
                    pr[:, ts(tci, E)],
                    lhsT=xr[:, d, :],
                    rhs=gw_sb[:, d, :],
                    start=(d == 0),
                    stop=(d == DO - 1),
                )

        def _softmax(pl_psum, n_c, tagp):
            """softmax over last axis of [P, n_c, E] logits in PSUM -> SBUF."""
            pl = smp.tile([P, n_c, E], F32, tag=tagp + "pl")
            nc.scalar.copy(pl[:], pl_psum.rearrange("p (t e) -> p t e", e=E))
            lmax = smp.tile([P, n_c, 1], F32, tag=tagp + "lmax")
            nc.vector.tensor_reduce(lmax[:], pl[:], axis=AX, op=OP.max)
            sub = smp.tile([P, n_c, E], F32, tag=tagp + "sub")
            nc.vector.tensor_tensor(
                sub[:], pl[:], lmax[:].to_broadcast([P, n_c, E]), op=OP.subtract
            )
            ex = smp.tile([P, n_c, E], F32, tag=tagp + "ex")
            nc.scalar.activation(ex[:], sub[:], ACTF.Exp)
            ssum = smp.tile([P, n_c, 1], F32, tag=tagp + "ssum")
            nc.vector.tensor_reduce(ssum[:], ex[:], axis=AX, op=OP.add)
            srec = smp.tile([P, n_c, 1], F32, tag=tagp + "srec")
            nc.vector.reciprocal(srec[:], ssum[:])
            pn = smp.tile([P, n_c, E], F32, tag=tagp + "pn")
            nc.vector.tensor_tensor(
                pn[:], ex[:], srec[:].to_broadcast([P, n_c, E]), op=OP.mult
            )
            return pn

        pn_full = _softmax(pr[:], n_tc, "f")
        nc.sync.dma_start(probs_r[:, :, :], pn_full[:])

        # ---------------- router on gathered tokens -> combine weights ----
        prg = psum.tile([P, n_gc * E], F32, tag="pp")
        for gci in range(n_gc):
            for d in range(DO):
                nc.tensor.matmul(
                    prg[:, ts(gci, E)],
                    lhsT=xg_sb[:, d, ts(gci, P)],
                    rhs=gwr_sb[:, d, :],
                    start=(d == 0),
                    stop=(d == DO - 1),
                )
        pg = _softmax(prg[:], n_gc, "g")
        m1 = smp.tile([P, n_gc, 1], F32, tag="m1")
        nc.vector.tensor_reduce(m1[:], pg[:], axis=AX, op=OP.max)
        # mask out the max to find the 2nd max
        msk = smp.tile([P, n_gc, E], F32, tag="msk")
        nc.vector.tensor_tensor(
            msk[:], pg[:], m1[:].to_broadcast([P, n_gc, E]), op=OP.is_equal
        )
        nc.vector.tensor_scalar_mul(msk[:], msk[:], -1e30)
        nc.vector.tensor_tensor(msk[:], msk[:], pg[:], op=OP.add)
        m2 = smp.tile([P, n_gc, 1], F32, tag="m2")
        nc.vector.tensor_reduce(m2[:], msk[:], axis=AX, op=OP.max)
        den = smp.tile([P, n_gc, 1], F32, tag="den")
        nc.vector.tensor_tensor(den[:], m1[:], m2[:], op=OP.add)
        rden = smp.tile([P, n_gc, 1], F32, tag="rden")
        nc.vector.reciprocal(rden[:], den[:])
        # own expert's prob: sum(p * onehot_e)
        pse = smp.tile([P, n_gc, E], F32, tag="pse")
        nc.vector.tensor_tensor(
            pse[:], pg[:], sel_sb[:].to_broadcast([P, n_gc, E]), op=OP.mult
        )
        cmb = cmbp.tile([P, n_gc, 1], F32, tag="cmb")
        nc.vector.tensor_reduce(cmb[:], pse[:], axis=AX, op=OP.add)
        nc.vector.tensor_tensor(cmb[:], cmb[:], rden[:], op=OP.mult)

        # ---------------- expert FFN ----------------
        for (toff, tlen) in blocks:
            ngb = tlen // P
            tslabs = _slabs(tlen)
            g_sb = gp.tile([P, NDF, tlen], F32R, tag="g")

            # h1 = x @ w_in, h2 = x @ w_v  (kept transposed: [DF, tokens])
            for wg in range(DF // WSLAB):
                win_t = winp.tile([P, DO, WSLAB], F32R, tag="win")
                nc.sync.dma_start(win_t[:], win_r[:, :, ts(wg, WSLAB)])
                wv_t = wvp.tile([P, DO, WSLAB], F32R, tag="wv")
                nc.sync.dma_start(wv_t[:], wv_r[:, :, ts(wg, WSLAB)])
                for dfi in range(WSLAB // P):
                    df = wg * (WSLAB // P) + dfi
                    for (so, sw) in tslabs:
                        ph1 = psum.tile([P, 512], F32, tag="pp")
                        ph2 = psum.tile([P, 512], F32, tag="pp")
                        for d in range(DO):
                            nc.tensor.matmul(
                                ph1[:, :sw],
                                lhsT=win_t[:, d, ts(dfi, P)],
                                rhs=xg_sb[:, d, ds(toff + so, sw)],
                                start=(d == 0),
                                stop=(d == DO - 1),
                            )
                        for d in range(DO):
                            nc.tensor.matmul(
                                ph2[:, :sw],
                                lhsT=wv_t[:, d, ts(dfi, P)],
                                rhs=xg_sb[:, d, ds(toff + so, sw)],
                                start=(d == 0),
                                stop=(d == DO - 1),
                            )
                        gel = gelp.tile([P, 512], F32, tag="gel")
                        nc.scalar.activation(gel[:, :sw], ph1[:, :sw], gelu_func)
                        nc.vector.tensor_tensor(
                            g_sb[:, df, ds(so, sw)], gel[:, :sw], ph2[:, :sw],
                            op=OP.mult,
                        )

            # out = (g.T @ w_out) * combine
            for dsi in range(NDS):
                pouts = [
                    psum.tile([P, 512], F32, tag="pp", name=f"pout{i}")
                    for i in range(ngb)
                ]
                for df in range(NDF):
                    wo_t = woutp.tile([P, 512], F32R, tag="wo")
                    nc.sync.dma_start(wo_t[:], wout_r[:, df, ts(dsi, 512)])
                    for tcl in range(ngb):
                        nc.tensor.matmul(
                            pouts[tcl][:],
                            lhsT=g_sb[:, df, ts(tcl, P)],
                            rhs=wo_t[:],
                            start=(df == 0),
                            stop=(df == NDF - 1),
                        )
                for tcl in range(ngb):
                    gci = toff // P + tcl
                    ot = outp.tile([P, 512], F32, tag="ot")
                    nc.vector.tensor_scalar_mul(ot[:], pouts[tcl][:], cmb[:, gci, :])
                    nc.sync.dma_start(
                        out[ds(toff + tcl * P, P), ts(dsi, 512)], ot[:]
                    )

    return nc


_BUILT = {}


def _get_nc(C):
    if C not in _BUILT:
        nc = bacc.Bacc("TRN2", num_devices=NCORES, debug=False)
        build_moe(nc, C)
        nc.compile()
        _BUILT[C] = nc
    return _BUILT[C]


def _round_f32r(a):
    """Round fp32 -> float32r (11 mantissa bits, low 12 bits zero, RNE)."""
    u = np.ascontiguousarray(a, np.float32).view(np.uint32)
    r = (u + np.uint32(0x7FF) + ((u >> np.uint32(12)) & np.uint32(1))) & np.uint32(
        0xFFFFF000
    )
    return r.view(np.float32)


def _route_host(xf, gate_w):
    """Host-side top-2 routing decisions (indices only)."""
    logits = xf @ gate_w
    order = np.argsort(-logits, axis=-1, kind="stable")
    top2 = order[:, :TOPK]
    idxs = [np.nonzero((top2 == e).any(axis=1))[0] for e in range(E)]
    return idxs


def _make_in_maps(xf, gate_w, w_in, w_v, w_out, idxs, C):
    xT = np.ascontiguousarray(xf.T)
    gw = np.ascontiguousarray(gate_w)
    gwr = _round_f32r(gate_w)
    in_maps = []
    for e in range(E):
        idx = idxs[e]
        pad = np.zeros(C, dtype=np.int64)
        pad[: len(idx)] = idx
        xg = _round_f32r(np.ascontiguousarray(xf[pad].T))
        sel_e = np.zeros((P, 1, E), np.float32)
        sel_e[:, 0, e] = 1.0
        in_maps.append(
            {
                "xT": xT,
                "xg": xg,
                "w_in": _round_f32r(w_in[e]),
                "w_v": _round_f32r(w_v[e]),
                "w_out": _round_f32r(w_out[e]),
                "gate_w": gw,
                "gate_wr": gwr,
                "sel": sel_e,
            }
        )
    return in_maps


LAST_RESULT = None


def kernel(x, gate_w, w_in, w_v, w_out, _trace=False, _trace_kwargs=None):
    global LAST_RESULT
    x = np.ascontiguousarray(np.asarray(x, dtype=np.float32))
    gate_w = np.ascontiguousarray(np.asarray(gate_w, dtype=np.float32))
    w_in = np.ascontiguousarray(np.asarray(w_in, dtype=np.float32))
    w_v = np.ascontiguousarray(np.asarray(w_v, dtype=np.float32))
    w_out = np.ascontiguousarray(np.asarray(w_out, dtype=np.float32))

    xf = x.reshape(T, D)
    idxs = _route_host(xf, gate_w)
    maxc = max(len(i) for i in idxs)
    C = max(P, ((maxc + P - 1) // P) * P)

    nc = _get_nc(C)
    in_maps = _make_in_maps(xf, gate_w, w_in, w_v, w_out, idxs, C)
    kw = dict(_trace_kwargs or {})
    res = run_bass_kernel_spmd(
        nc, in_maps, core_ids=list(range(NCORES)), trace=_trace, **kw
    )
    LAST_RESULT = res

    output = np.zeros((T, D), np.float32)
    for e in range(E):
        n = len(idxs[e])
        output[idxs[e]] += res.results[e]["out"][:n]
    probs_out = np.asarray(res.results[0]["probs"])
    return output.reshape(B, S, D), probs_out.reshape(B, S, E)


# revision 18
# speedup vs baseline: 1.0233x; 1.0233x over previous
"""Expert-parallel MoE (Grok-1-style dense-masked MoE layer) for Trainium2.

Strategy (8 NeuronCores, expert-parallel):
  - Router top-2 decisions are computed on the host (fp32, verified to match
    the jax reference's decisions exactly on margins >> fp error), and used
    ONLY to build a per-expert gathered token list (capacity-padded).
  - Each core owns one expert e: it receives x gathered for its tokens
    (xg = x[idx_e].T), the expert's weights, and the full xT (for the
    router-probs output, computed on device).
  - On device, each core computes:
      * router probs over ALL tokens (softmax(x @ gate_w)) -> probs output
        (host keeps core 0's copy),
      * the top-2-renormalized combine weight for its own expert on its
        gathered tokens (softmax + 2nd-max + renorm, all on-chip),
      * h1 = xg @ w_in[e], h2 = xg @ w_v[e]  (f32r matmuls, contraction
        over D on partitions),
      * g = gelu(h1) * h2 kept on SBUF in [DF, tokens] layout,
      * out = (g.T @ w_out[e]) * combine  -> DMA to HBM.
  - Host unshard: output[idx_e] += core_e_out[:count_e]  (the all-reduce
    over experts collapses to a scatter-add since non-routed tokens
    contribute exactly zero).
"""

import numpy as np
from contextlib import ExitStack

import concourse.bass as bass
import concourse.tile as tile
from concourse import bacc, mybir
from concourse.bass import ts, ds
from concourse.bass_utils import run_bass_kernel_spmd

B, S, D, DF, E, TOPK = 2, 1024, 1024, 4096, 8, 2
T = B * S
P = 128
NCORES = 8

F32 = mybir.dt.float32
F32R = mybir.dt.float32r
AX = mybir.AxisListType.X
OP = mybir.AluOpType
ACTF = mybir.ActivationFunctionType


def _slabs(n, mx=512):
    """Split n into contiguous slabs each <= mx, as even as possible."""
    k = -(-n // mx)
    w = -(-n // k)
    out = []
    o = 0
    while o < n:
        s = min(w, n - o)
        out.append((o, s))
        o += s
    return out


def build_moe(nc, C, gelu_func=None):
    """Emit the per-core kernel IR. C = gathered-token capacity (mult of 128)."""
    if gelu_func is None:
        gelu_func = ACTF.Gelu
    assert C % P == 0
    n_tc = T // P      # full-router token chunks (16)
    n_gc = C // P      # gathered token chunks
    DO = D // P        # 8 contraction chunks over D
    NDF = DF // P      # 32 chunks over DF
    NDS = D // 512     # 2 output D slabs

    xg = nc.dram_tensor("xg", [D, C], F32R, kind="ExternalInput").ap()
    w_in = nc.dram_tensor("w_in", [D, DF], F32R, kind="ExternalInput").ap()
    w_v = nc.dram_tensor("w_v", [D, DF], F32R, kind="ExternalInput").ap()
    w_out = nc.dram_tensor("w_out", [DF, D], F32R, kind="ExternalInput").ap()
    gate_wr = nc.dram_tensor("gate_wr", [D, E], F32R, kind="ExternalInput").ap()
    sel = nc.dram_tensor("sel", [P, 1, E], F32, kind="ExternalInput").ap()
    out = nc.dram_tensor("out", [C, D], F32, kind="ExternalOutput").ap()
    probs = nc.dram_tensor("probs", [C, E], F32, kind="ExternalOutput").ap()

    xg_r = xg.rearrange("(o p) t -> p o t", p=P)
    win_r = w_in.rearrange("(o p) f -> p o f", p=P)
    wv_r = w_v.rearrange("(o p) f -> p o f", p=P)
    wout_r = w_out.rearrange("(o p) d -> p o d", p=P)
    gwr_r = gate_wr.rearrange("(o p) e -> p o e", p=P)
    probs_r = probs.rearrange("(t p) e -> p t e", p=P)

    WSLAB = 256 if C <= 1024 else 128
    # token blocks sized so g ([P, NDF, tlen] fp32) fits in SBUF
    if C <= 640:
        blocks = [(0, C)]
    else:
        blocks = [(o, min(512, C - o)) for o in range(0, C, 512)]

    with tile.TileContext(nc) as tc, ExitStack() as ctx:
        const = ctx.enter_context(tc.tile_pool(name="const", bufs=1))
        xgp = ctx.enter_context(tc.tile_pool(name="xgp", bufs=1))
        gp = ctx.enter_context(tc.tile_pool(name="gp", bufs=1))
        winp = ctx.enter_context(tc.tile_pool(name="winp", bufs=2))
        wvp = ctx.enter_context(tc.tile_pool(name="wvp", bufs=2))
        woutp = ctx.enter_context(tc.tile_pool(name="woutp", bufs=3))
        smp = ctx.enter_context(tc.tile_pool(name="smp", bufs=1))
        cmbp = ctx.enter_context(tc.tile_pool(name="cmbp", bufs=1))
        gelp = ctx.enter_context(tc.tile_pool(name="gelp", bufs=2))
        outp = ctx.enter_context(tc.tile_pool(name="outp", bufs=4))
        psum = ctx.enter_context(tc.tile_pool(name="psum", bufs=8, space="PSUM"))

        # ---------------- constants ----------------
        gwr_sb = const.tile([P, DO, E], F32R, tag="gwr")
        nc.sync.dma_start(gwr_sb[:], gwr_r[:, :, :])
        sel_sb = const.tile([P, 1, E], F32, tag="sel")
        nc.sync.dma_start(sel_sb[:], sel[:, :, :])

        # ---------------- gathered activations (resident) ----------------
        xg_sb = xgp.tile([P, DO, C], F32R, tag="xg")
        nc.sync.dma_start(xg_sb[:], xg_r[:, :, :])

        def _softmax(pl_psum, n_c, tagp):
            """softmax over last axis of [P, n_c, E] logits in PSUM -> SBUF."""
            pl = smp.tile([P, n_c, E], F32, tag=tagp + "pl")
            nc.scalar.copy(pl[:], pl_psum.rearrange("p (t e) -> p t e", e=E))
            lmax = smp.tile([P, n_c, 1], F32, tag=tagp + "lmax")
            nc.vector.tensor_reduce(lmax[:], pl[:], axis=AX, op=OP.max)
            sub = smp.tile([P, n_c, E], F32, tag=tagp + "sub")
            nc.vector.tensor_tensor(
                sub[:], pl[:], lmax[:].to_broadcast([P, n_c, E]), op=OP.subtract
            )
            ex = smp.tile([P, n_c, E], F32, tag=tagp + "ex")
            nc.scalar.activation(ex[:], sub[:], ACTF.Exp)
            ssum = smp.tile([P, n_c, 1], F32, tag=tagp + "ssum")
            nc.vector.tensor_reduce(ssum[:], ex[:], axis=AX, op=OP.add)
            srec = smp.tile([P, n_c, 1], F32, tag=tagp + "srec")
            nc.vector.reciprocal(srec[:], ssum[:])
            pn = smp.tile([P, n_c, E], F32, tag=tagp + "pn")
            nc.vector.tensor_tensor(
                pn[:], ex[:], srec[:].to_broadcast([P, n_c, E]), op=OP.mult
            )
            return pn

        # ---------------- router on gathered tokens -> combine weights ----
        prg = psum.tile([P, n_gc * E], F32, tag="pp")
        for gci in range(n_gc):
            for d in range(DO):
                nc.tensor.matmul(
                    prg[:, ts(gci, E)],
                    lhsT=xg_sb[:, d, ts(gci, P)],
                    rhs=gwr_sb[:, d, :],
                    start=(d == 0),
                    stop=(d == DO - 1),
                )
        pg = _softmax(prg[:], n_gc, "g")
        nc.sync.dma_start(probs_r[:, :, :], pg[:])
        m1 = smp.tile([P, n_gc, 1], F32, tag="m1")
        nc.vector.tensor_reduce(m1[:], pg[:], axis=AX, op=OP.max)
        # mask out the max to find the 2nd max
        msk = smp.tile([P, n_gc, E], F32, tag="msk")
        nc.vector.tensor_tensor(
            msk[:], pg[:], m1[:].to_broadcast([P, n_gc, E]), op=OP.is_equal
        )
        nc.vector.tensor_scalar_mul(msk[:], msk[:], -1e30)
        nc.vector.tensor_tensor(msk[:], msk[:], pg[:], op=OP.add)
        m2 = smp.tile([P, n_gc, 1], F32, tag="m2")
        nc.vector.tensor_reduce(m2[:], msk[:], axis=AX, op=OP.max)
        den = smp.tile([P, n_gc, 1], F32, tag="den")
        nc.vector.tensor_tensor(den[:], m1[:], m2[:], op=OP.add)
        rden = smp.tile([P, n_gc, 1], F32, tag="rden")
        nc.vector.reciprocal(rden[:], den[:])
        # own expert's prob: sum(p * onehot_e)
        pse = smp.tile([P, n_gc, E], F32, tag="pse")
        nc.vector.tensor_tensor(
            pse[:], pg[:], sel_sb[:].to_broadcast([P, n_gc, E]), op=OP.mult
        )
        cmb = cmbp.tile([P, n_gc, 1], F32, tag="cmb")
        nc.vector.tensor_reduce(cmb[:], pse[:], axis=AX, op=OP.add)
        nc.vector.tensor_tensor(cmb[:], cmb[:], rden[:], op=OP.mult)

        # ---------------- expert FFN ----------------
        for (toff, tlen) in blocks:
            ngb = tlen // P
            tslabs = _slabs(tlen)
            g_sb = gp.tile([P, NDF, tlen], F32R, tag="g")

            # h1 = x @ w_in, h2 = x @ w_v  (kept transposed: [DF, tokens])
            for wg in range(DF // WSLAB):
                win_t = winp.tile([P, DO, WSLAB], F32R, tag="win")
                nc.sync.dma_start(win_t[:], win_r[:, :, ts(wg, WSLAB)])
                wv_t = wvp.tile([P, DO, WSLAB], F32R, tag="wv")
                nc.sync.dma_start(wv_t[:], wv_r[:, :, ts(wg, WSLAB)])
                for dfi in range(WSLAB // P):
                    df = wg * (WSLAB // P) + dfi
                    ph1 = [
                        psum.tile([P, 512], F32, tag="pp", name=f"ph1_{i}")
                        for i in range(len(tslabs))
                    ]
                    ph2 = [
                        psum.tile([P, 512], F32, tag="pp", name=f"ph2_{i}")
                        for i in range(len(tslabs))
                    ]
                    for d in range(DO):
                        for si, (so, sw) in enumerate(tslabs):
                            nc.tensor.matmul(
                                ph1[si][:, :sw],
                                lhsT=win_t[:, d, ts(dfi, P)],
                                rhs=xg_sb[:, d, ds(toff + so, sw)],
                                start=(d == 0),
                                stop=(d == DO - 1),
                            )
                    for d in range(DO):
                        for si, (so, sw) in enumerate(tslabs):
                            nc.tensor.matmul(
                                ph2[si][:, :sw],
                                lhsT=wv_t[:, d, ts(dfi, P)],
                                rhs=xg_sb[:, d, ds(toff + so, sw)],
                                start=(d == 0),
                                stop=(d == DO - 1),
                            )
                    for si, (so, sw) in enumerate(tslabs):
                        gel = gelp.tile([P, 512], F32, tag="gel")
                        nc.scalar.activation(gel[:, :sw], ph1[si][:, :sw], gelu_func)
                        nc.vector.tensor_tensor(
                            g_sb[:, df, ds(so, sw)], gel[:, :sw], ph2[si][:, :sw],
                            op=OP.mult,
                        )

            # out = (g.T @ w_out) * combine
            tgroups = [list(range(0, min(4, ngb)))]
            if ngb > 4:
                tgroups.append(list(range(4, ngb)))
            for tgrp in tgroups:
                pouts = {
                    (tcl, dsi): psum.tile(
                        [P, 512], F32, tag="pp", name=f"pout{tcl}_{dsi}"
                    )
                    for tcl in tgrp
                    for dsi in range(NDS)
                }
                for df in range(NDF):
                    wo_t = woutp.tile([P, D], F32R, tag="wo")
                    nc.sync.dma_start(wo_t[:], wout_r[:, df, :])
                    for tcl in tgrp:
                        for dsi in range(NDS):
                            nc.tensor.matmul(
                                pouts[(tcl, dsi)][:],
                                lhsT=g_sb[:, df, ts(tcl, P)],
                                rhs=wo_t[:, ts(dsi, 512)],
                                start=(df == 0),
                                stop=(df == NDF - 1),
                            )
                for tcl in tgrp:
                    gci = toff // P + tcl
                    for dsi in range(NDS):
                        ot = outp.tile([P, 512], F32, tag="ot")
                        nc.vector.tensor_scalar_mul(
                            ot[:], pouts[(tcl, dsi)][:], cmb[:, gci, :]
                        )
                        nc.sync.dma_start(
                            out[ds(toff + tcl * P, P), ts(dsi, 512)], ot[:]
                        )

    return nc


_BUILT = {}


def _get_nc(C):
    if C not in _BUILT:
        nc = bacc.Bacc("TRN2", num_devices=NCORES, debug=False)
        build_moe(nc, C)
        nc.compile()
        _BUILT[C] = nc
    return _BUILT[C]


def _round_f32r(a):
    """Round fp32 -> float32r (11 mantissa bits, low 12 bits zero, RNE)."""
    u = np.ascontiguousarray(a, np.float32).view(np.uint32)
    r = (u + np.uint32(0x7FF) + ((u >> np.uint32(12)) & np.uint32(1))) & np.uint32(
        0xFFFFF000
    )
    return r.view(np.float32)


def _route_host(xf, gate_w):
    """Host-side top-2 routing decisions (indices only)."""
    logits = xf @ gate_w
    order = np.argsort(-logits, axis=-1, kind="stable")
    top2 = order[:, :TOPK]
    idxs = [np.nonzero((top2 == e).any(axis=1))[0] for e in range(E)]
    return idxs


def _make_in_maps(xf, gate_w, w_in, w_v, w_out, idxs, C):
    gwr = _round_f32r(gate_w)
    in_maps = []
    for e in range(E):
        idx = idxs[e]
        pad = np.zeros(C, dtype=np.int64)
        pad[: len(idx)] = idx
        xg = _round_f32r(np.ascontiguousarray(xf[pad].T))
        sel_e = np.zeros((P, 1, E), np.float32)
        sel_e[:, 0, e] = 1.0
        in_maps.append(
            {
                "xg": xg,
                "w_in": _round_f32r(w_in[e]),
                "w_v": _round_f32r(w_v[e]),
                "w_out": _round_f32r(w_out[e]),
                "gate_wr": gwr,
                "sel": sel_e,
            }
        )
    return in_maps


LAST_RESULT = None


def kernel(x, gate_w, w_in, w_v, w_out, _trace=False, _trace_kwargs=None):
    global LAST_RESULT
    x = np.ascontiguousarray(np.asarray(x, dtype=np.float32))
    gate_w = np.ascontiguousarray(np.asarray(gate_w, dtype=np.float32))
    w_in = np.ascontiguousarray(np.asarray(w_in, dtype=np.float32))
    w_v = np.ascontiguousarray(np.asarray(w_v, dtype=np.float32))
    w_out = np.ascontiguousarray(np.asarray(w_out, dtype=np.float32))

    xf = x.reshape(T, D)
    idxs = _route_host(xf, gate_w)
    maxc = max(len(i) for i in idxs)
    C = max(P, ((maxc + P - 1) // P) * P)

    nc = _get_nc(C)
    in_maps = _make_in_maps(xf, gate_w, w_in, w_v, w_out, idxs, C)
    kw = dict(_trace_kwargs or {})
    res = run_bass_kernel_spmd(
        nc, in_maps, core_ids=list(range(NCORES)), trace=_trace, **kw
    )
    LAST_RESULT = res

    output = np.zeros((T, D), np.float32)
    probs_out = np.zeros((T, E), np.float32)
    for e in range(E):
        n = len(idxs[e])
        output[idxs[e]] += res.results[e]["out"][:n]
        probs_out[idxs[e]] = res.results[e]["probs"][:n]
    return output.reshape(B, S, D), probs_out.reshape(B, S, E)
